# revision 1
# baseline (speedup 1.0000x reference)
"""Trainium2 Bass kernel for nn_Block1_87144886436577 (vq_codebook).

The reference's giant jacobians (jacrev through a 2-layer strided conv
net) collapse analytically: each output pixel o sees a 10x10 input
patch, so the per-o gradient image e_total[b,o] is a 10x10 patch
computed with tap matmuls; the argmin reduces to a 9-candidate compare
plus the structural-zero tie rule; the scatter-accumulated y_masked
collapses to tap matmuls over a selection-masked x.

v2: the 16-tap convs collapse to 4 K=128 matmuls each by baking the
k1x/k2x shifts into 4 partition-block replicas of the input image
(xp4/y1p4), so conv1/conv2/ep/ym need no PE tiling; index tables are
host-arranged for single contiguous DMAs. The e9 gather data uses
partition order (b, r, h, q) with ep2 scattered into 8 pre-shifted
DRAM copies per batch (h/e shifts baked into copy bases, 16-slot-wide
rows so invalid candidates hit an h-independent zero pad), letting
data_e load as two 64-partition-wide DMAs with no doubling chain. The
conv1-critical chain (w1 taps, shifted x blocks) issues first on the
sync DMA FIFO; bulk const/codebook loads go on the scalar FIFO or
after the forward pass, since HWDGE FIFOs drain in program order.

Single-core program; all 8 cores run identical replicas. Output read
from core 0.
"""
import sys

import numpy as np

for _p in ("/opt/trn_rl_repo",):
    if _p not in sys.path:
        sys.path.insert(0, _p)

import concourse.bass as bass
import concourse.mybir as mybir
import concourse.tile as tile

F32 = mybir.dt.float32
U16 = mybir.dt.uint16
AF = mybir.ActivationFunctionType
ALU = mybir.AluOpType
AX = mybir.AxisListType
AP = bass.AP

N_CORES = 8


def v(t, off, pat):
    return AP(t.tensor, t.offset + off, pat)


def _e(r):
    return 1 if r >= 1 else 0


def _consts():
    """Host-precomputed constant tensors (input-independent)."""
    ident128 = np.eye(128, dtype=np.float32)
    oidx128 = np.tile((np.arange(128) % 64).astype(np.float32)[None, :],
                      (128, 1))

    idxX = np.zeros((8, 128, 8), np.uint16)
    for t in range(8):
        k2y, k2xh = t // 2, t % 2
        for g in range(8):
            k2xp = g // 4
            k1y = g % 4
            k2x = 2 * k2xh + k2xp
            for j in range(128):
                b, oy, ox = j // 64, (j % 64) // 8, j % 8
                idxX[t, 16 * g + j % 16, j // 16] = (
                    b * 1444 + (4 * oy + 2 * k2y + k1y) * 38
                    + 4 * ox + 2 * k2x)
    idxXs = np.ascontiguousarray(
        idxX.transpose(1, 0, 2).reshape(128, 64))

    idxS = np.zeros((2, 128, 8), np.uint16)
    for s in range(2):
        for g in range(8):
            k2yp, k2x = g // 4, g % 4
            k2y = s * 2 + k2yp
            for j in range(128):
                b, oy, ox = j // 64, (j % 64) // 8, j % 8
                idxS[s, 16 * g + j % 16, j // 16] = (
                    b * 1444 + (4 * oy + 2 * k2y) * 38 + 4 * ox + 2 * k2x)
    idxSs = np.ascontiguousarray(
        idxS.transpose(1, 0, 2).reshape(128, 16))

    emat = np.zeros((8, 128, 128), np.float32)
    for t in range(8):
        k2y, k2xh = t // 2, t % 2
        for p in range(128):
            k2xp, k1y, k1x = p // 64, (p % 64) // 16, p % 4
            k2x = 2 * k2xh + k2xp
            r = (k2y % 2) * 64 + k2x * 16 + k1y * 4 + k1x
            emat[t, r, p] = 1.0
    emats = np.ascontiguousarray(
        emat.transpose(1, 0, 2).reshape(128, 1024))

    # groups are (b, r); h is a partition dim whose shift is baked
    # into the DRAM copy bases, so idx uses oxp' = oxp - 4h (h-free)
    idxE = np.zeros((128, 9), np.uint16)
    for b in range(2):
        for r in range(4):
            g = b * 4 + r
            for j in range(144):
                ixl, jj = j // 9, j % 9
                jy, jx = jj // 3, jj % 3
                t_ = ixl % 4
                dy = r - 4 * _e(r) + 4 * jy + 3
                dx = t_ - 4 * _e(t_) + 4 * jx + 3
                oxpp = ixl // 4 + _e(t_) - jx + 2
                if 0 <= dy < 10 and 0 <= dx < 10:
                    idx = (2 - jy) * 1600 + oxpp * 100 + dy * 10 + dx
                else:
                    idx = 2 * 1600 + 10 * 100  # pad col: always 0
                idxE[16 * g + j % 16, j // 16] = idx

    oidx9 = np.full((128, 144), 3000.0, np.float32)
    zc = np.zeros((128, 16), np.float32)
    for r in range(4):
        for h in range(2):
            for b in range(2):
                for q in range(8):
                    p = b * 64 + r * 16 + h * 8 + q
                    iy = 4 * q + r
                    for ixl in range(16):
                        ix = 16 * h + ixl
                        t_ = ix % 4
                        s = ix // 4
                        for jj in range(9):
                            jy, jx = jj // 3, jj % 3
                            oy = q + _e(r) - jy
                            ox = s + _e(t_) - jx
                            dy = iy - 4 * oy + 3
                            dx = ix - 4 * ox + 3
                            if (0 <= oy < 8 and 0 <= ox < 8
                                    and 0 <= dy < 10 and 0 <= dx < 10):
                                oidx9[p, ixl * 9 + jj] = oy * 8 + ox
                        for o in range(64):
                            oy, ox = o // 8, o % 8
                            if not (0 <= iy - 4 * oy + 3 < 10
                                    and 0 <= ix - 4 * ox + 3 < 10):
                                zc[p, ixl] = float(o)
                                break
    return {"ident128": ident128, "oidx128": oidx128,
            "idxXs": idxXs, "idxSs": idxSs, "emats": emats,
            "idxE": idxE, "oidx9": oidx9, "zc": zc}


def build_program(nc):
    x_d = nc.declare_dram_parameter("x", [2, 3, 32, 32], F32, isOutput=False)
    w1_d = nc.declare_dram_parameter("w1", [32, 3, 4, 4], F32, isOutput=False)
    b1_d = nc.declare_dram_parameter("b1", [32], F32, isOutput=False)
    w2_d = nc.declare_dram_parameter("w2", [64, 32, 4, 4], F32, isOutput=False)
    b2_d = nc.declare_dram_parameter("b2", [64], F32, isOutput=False)
    k_d = nc.declare_dram_parameter("K", [512, 64], F32, isOutput=False)
    v_d = nc.declare_dram_parameter("V", [512, 64], F32, isOutput=False)
    id_d = nc.declare_dram_parameter("ident128", [128, 128], F32,
                                     isOutput=False)
    oi_d = nc.declare_dram_parameter("oidx128", [128, 128], F32,
                                     isOutput=False)
    ixx_d = nc.declare_dram_parameter("idxXs", [128, 64], U16,
                                      isOutput=False)
    ixs_d = nc.declare_dram_parameter("idxSs", [128, 16], U16,
                                      isOutput=False)
    em_d = nc.declare_dram_parameter("emats", [128, 1024], F32,
                                     isOutput=False)
    ixe_d = nc.declare_dram_parameter("idxE", [128, 9], U16, isOutput=False)
    oi9_d = nc.declare_dram_parameter("oidx9", [128, 144], F32,
                                      isOutput=False)
    zc_d = nc.declare_dram_parameter("zc", [128, 16], F32, isOutput=False)
    out_d = nc.declare_dram_parameter("out", [2, 64, 8, 8], F32,
                                      isOutput=True)

    with tile.TileContext(nc) as tc:
        with (
            tc.tile_pool(name="const", bufs=1) as cpool,
            tc.tile_pool(name="work", bufs=1) as wpool,
            tc.tile_pool(name="psA", bufs=2, space="PSUM") as psA,
            tc.tile_pool(name="psB", bufs=2, space="PSUM") as psB,
            tc.tile_pool(name="psC", bufs=1, space="PSUM") as psC,
            tc.tile_pool(name="dram", bufs=1, space="DRAM") as dpool,
        ):
            dma = nc.sync.dma_start

            # ---- DRAM scratch ----
            # x_pad2: [ci][b][38x38], 4 ci slots (last all-zero) + slop
            x_pad2 = dpool.tile([11680], F32)
            sel_pad = dpool.tile([3040], F32)
            edR = dpool.tile([256000], F32)

            z128 = cpool.tile([128, 2400], F32)
            nc.gpsimd.memset(z128[:], 0.0)
            zneg = cpool.tile([2, 1520], F32)
            nc.gpsimd.memset(zneg[:], -1.0)
            for i in range(2):
                nc.scalar.dma_start(
                    v(edR, i * 128000, [[2000, 64], [1, 2000]]),
                    z128[0:64, 0:2000])
            nc.scalar.dma_start(v(sel_pad, 0, [[1520, 2], [1, 1520]]),
                                zneg[:])
            nc.scalar.dma_start(v(x_pad2, 0, [[2336, 5], [1, 2336]]),
                                z128[0:5, 0:2336])

            # conv1-critical chain first: w1 taps + shifted x blocks
            ident32 = cpool.tile([32, 32], F32)
            dma(ident32[:], AP(id_d, 0, [[128, 32], [1, 32]]))
            w1sb = wpool.tile([32, 48], F32)          # [m, (ci,k1)]
            dma(w1sb[:], AP(w1_d, 0, [[48, 32], [1, 48]]))
            xp4 = wpool.tile([128, 2888], F32)
            xpp = xp4.ap[0][0]
            nc.vector.memset(xp4[:], 0.0)
            for l in range(4):
                for b in range(2):
                    eng = dma if (2 * l + b) % 2 == 0 \
                        else nc.scalar.dma_start
                    eng(v(xp4, 32 * l * xpp + b * 1444 + 117 - l,
                          [[xpp, 3], [38, 32], [1, 32]]),
                        AP(x_d, b * 3072, [[1024, 3], [32, 32], [1, 32]]))
            w1sbP = wpool.tile([32, 48], F32)
            nc.vector.tensor_copy(
                v(w1sbP, 0, [[w1sbP.ap[0][0], 32], [3, 16], [1, 3]]),
                v(w1sb, 0, [[w1sb.ap[0][0], 32], [1, 16], [16, 3]]))
            wb_ps = psB.tile([48, 32], F32, tag="psB", name="wb_ps")
            nc.tensor.transpose(wb_ps[:], w1sbP[:], ident32[:])
            wB = wpool.tile([48, 32], F32)
            nc.scalar.copy(wB[:], wb_ps[:])
            w1g = []
            for g in range(4):
                wg = wpool.tile([128, 32], F32, name=f"w1g{g}")
                nc.vector.memset(wg[:], 0.0)
                for l in range(4):
                    k1 = 4 * g + l
                    eng = dma if l % 2 == 0 else nc.scalar.dma_start
                    eng(wg[32 * l:32 * l + 3, :],
                        wB[3 * k1:3 * k1 + 3, :])
                w1g.append(wg)
            ixX = cpool.tile([128, 64], U16)
            dma(ixX[:], ixx_d[:])
            ident = cpool.tile([128, 128], F32)
            dma(ident[:], id_d[:])

            # zero-fills for DRAM scratch


            # ---- padded x image in DRAM: [ci][b][38][38] ----
            for b in range(2):
                nc.scalar.dma_start(
                    v(x_pad2, b * 1444 + 117,
                      [[2888, 3], [38, 32], [1, 32]]),
                    AP(x_d, b * 3072, [[1024, 3], [32, 32], [1, 32]]))
            # ---- weight staging ----
            # w2sb [m, (c,k2)] then w2s4 [128=(l,m), 256=(g,c)] via copies
            w2sb = wpool.tile([32, 1024], F32)
            dma(w2sb[:], AP(w2_d, 0, [[16, 32], [512, 64], [1, 16]]))
            w2s4 = wpool.tile([128, 256], F32)
            for l in range(4):
                nc.vector.tensor_copy(
                    v(w2s4, 32 * l * w2s4.ap[0][0],
                      [[w2s4.ap[0][0], 32], [64, 4], [1, 64]]),
                    v(w2sb, l, [[w2sb.ap[0][0], 32], [4, 4], [16, 64]]))
            b1t4 = wpool.tile([128, 1], F32)
            for j in range(4):
                dma(b1t4[j * 32:(j + 1) * 32, :],
                    AP(b1_d, 0, [[1, 32], [1, 1]]))
            b2t = wpool.tile([64, 1], F32)
            dma(b2t[:], AP(b2_d, 0, [[1, 64], [1, 1]]))




            # x-side gather data (setup; overlaps phases A/B)
            data_x = wpool.tile([128, 2888], F32)
            for ci in range(4):
                nc.scalar.dma_start(data_x[ci * 4:ci * 4 + 4, :],
                    v(x_pad2, ci * 2888, [[1, 4], [1, 2888]]))
            for d in (16, 32, 64):
                nc.scalar.dma_start(data_x[d:2 * d, :], data_x[0:d, :])
            xg3 = []
            for t in range(8):
                xg = wpool.tile([128, 128], F32, name=f"xg{t}")
                nc.gpsimd.indirect_copy(
                    v(xg, 0, [[xg.ap[0][0], 128], [1, 128], [1, 1]]),
                    data_x[:], ixX[:, t * 8:(t + 1) * 8], True)
                xg3.append(xg)

            # ---- Phase A: forward ----
            # conv1: 4 K=128 matmuls (taps K-stacked via shifted blocks)
            y1ps = psA.tile([32, 512], F32, tag="psA", name="y1ps")
            for g in range(4):
                nc.tensor.matmul(
                    y1ps[:], w1g[g][:],
                    v(xp4, 78 + 38 * g, [[xpp, 128], [1444, 2],
                                         [76, 16], [2, 16]]),
                    start=(g == 0), stop=(g == 3))
            # y1p4: 4 blocks, block l pre-shifted by k2x=l (written at
            # origin 19-l) so conv2/g1/ym K-stack their taps
            y1p4 = wpool.tile([128, 648], F32)
            nc.vector.memset(y1p4[:], 0.0)
            ypitch = y1p4.ap[0][0]
            nc.scalar.activation(
                v(y1p4, 19, [[ypitch, 32], [324, 2], [18, 16], [1, 16]]),
                y1ps[:], AF.Relu, bias=b1t4[0:32, :])
            iview = [[ypitch, 32], [324, 2], [18, 16], [1, 16]]
            for l in range(1, 4):
                nc.vector.tensor_copy(
                    v(y1p4, 32 * l * ypitch + 19 - l, iview),
                    v(y1p4, 19, iview))
            m1p4 = wpool.tile([128, 648], F32)
            nc.vector.tensor_scalar(m1p4[:], y1p4[:], 0.0, None, ALU.is_gt)

            def tapg(tl, g, pitch, np_=128):
                # group-g tap view across all 4 shift-baked blocks
                return v(tl, 18 * g,
                         [[pitch, np_], [324, 2], [36, 8], [2, 8]])

            # conv2: 4 K=128 matmuls (4 k2x-taps stacked per group)
            ypre = psA.tile([64, 128], F32, tag="psA", name="ypre")
            for g in range(4):
                nc.tensor.matmul(
                    ypre[:],
                    v(w2s4, 64 * g, [[w2s4.ap[0][0], 128], [1, 64]]),
                    tapg(y1p4, g, ypitch),
                    start=(g == 0), stop=(g == 3))
            yT = wpool.tile([64, 128], F32)    # [c, (b,o)]
            nc.scalar.activation(yT[:], ypre[:], AF.Relu, bias=b2t[:])
            m2T = wpool.tile([64, 128], F32)
            nc.vector.tensor_scalar(m2T[:], yT[:], 0.0, None, ALU.is_gt)
            # K^T and V in SBUF
            kt_sb = wpool.tile([64, 512], F32)
            v_sb = wpool.tile([128, 256], F32)
            for t in range(4):
                k_tile = wpool.tile([128, 64], F32, tag="k_tile",
                                    name="k_tile")
                dma(k_tile[:], AP(k_d, t * 8192, [[64, 128], [1, 64]]))
                kt_ps = psB.tile([64, 128], F32, tag="psB", name="kt_ps")
                nc.tensor.transpose(kt_ps[:], k_tile[:], ident[:])
                nc.scalar.copy(kt_sb[:, t * 128:(t + 1) * 128], kt_ps[:])
                nc.scalar.dma_start(v_sb[:, t * 64:(t + 1) * 64],
                    AP(v_d, t * 8192, [[64, 128], [1, 64]]))
            # w2c2s [128=(h,c), 512=(m,k2)] doubled halves
            w2c2s = wpool.tile([128, 512], F32)
            for h in range(2):
                dma(w2c2s[h * 64:(h + 1) * 64, :],
                    AP(w2_d, 0, [[512, 64], [16, 32], [1, 16]]))
            # w1fp [128, 32]: rows x2 of (k1y,ci4,k1x)->m
            w1sb2 = wpool.tile([32, 48], F32)
            dma(w1sb2[:], AP(w1_d, 0, [[48, 32], [16, 3], [1, 16]]))
            w1sb2p = wpool.tile([32, 64], F32)
            nc.vector.memset(w1sb2p[:], 0.0)
            nc.vector.tensor_copy(
                v(w1sb2p, 0, [[w1sb2p.ap[0][0], 32], [16, 4], [4, 3], [1, 4]]),
                v(w1sb2, 0, [[w1sb2.ap[0][0], 32], [4, 4], [16, 3], [1, 4]]))
            w1fp_ps = psB.tile([64, 32], F32, tag="psB", name="w1fp_ps")
            nc.tensor.transpose(w1fp_ps[:], w1sb2p[:], ident[0:32, 0:32])
            w1fp = wpool.tile([128, 32], F32)
            nc.scalar.copy(w1fp[0:64, :], w1fp_ps[:])
            dma(w1fp[64:128, :], w1fp[0:64, :])

            # deferred const loads
            oidx = cpool.tile([128, 128], F32)
            nc.scalar.dma_start(oidx[:], oi_d[:])
            ixS = cpool.tile([128, 16], U16)
            nc.scalar.dma_start(ixS[:], ixs_d[:])
            emt = cpool.tile([128, 1024], F32)
            nc.scalar.dma_start(emt[:], em_d[:])
            ixE = cpool.tile([128, 9], U16)
            nc.scalar.dma_start(ixE[:], ixe_d[:])
            oidx9 = cpool.tile([128, 144], F32)
            nc.scalar.dma_start(oidx9[:], oi9_d[:])
            zct = cpool.tile([128, 16], F32)
            nc.scalar.dma_start(zct[:], zc_d[:])

            def hopfield(src, tag):
                """src [c 64, (b,o) 128] -> yq [(b,o) 128, c 64] SBUF."""
                a_ps = psA.tile([128, 512], F32, tag="psA",
                                name=f"a_ps{tag}")
                nc.tensor.matmul(a_ps[:], src, kt_sb[:], start=True,
                                 stop=True)
                rmax = wpool.tile([128, 1], F32, name=f"rmax{tag}")
                nc.vector.tensor_reduce(rmax[:], a_ps[:], AX.X, ALU.max)
                negbm = wpool.tile([128, 1], F32, name=f"negbm{tag}")
                nc.vector.tensor_scalar(negbm[:], rmax[:], -0.125, None,
                                        ALU.mult)
                p_sb = wpool.tile([128, 512], F32, name=f"p_sb{tag}")
                ssum = wpool.tile([128, 1], F32, name=f"ssum{tag}")
                nc.scalar.activation(p_sb[:], a_ps[:], AF.Exp, bias=negbm[:],
                                     scale=0.125, accum_out=ssum[:])
                rec = wpool.tile([128, 1], F32, name=f"rec{tag}")
                nc.vector.reciprocal(rec[:], ssum[:])
                yq_ps = psC.tile([128, 64], F32, tag="psC",
                                 name=f"yq_ps{tag}")
                for t in range(4):
                    pt_ps = psB.tile([128, 128], F32, tag="psB",
                                     name=f"pt_ps{tag}{t}")
                    nc.tensor.transpose(pt_ps[:],
                                        p_sb[:, t * 128:(t + 1) * 128],
                                        ident[:])
                    pt_sb = wpool.tile([128, 128], F32, tag="pt_sb",
                                       name=f"pt_sb{tag}{t}")
                    nc.scalar.copy(pt_sb[:], pt_ps[:])
                    nc.tensor.matmul(yq_ps[:], pt_sb[:],
                                     v_sb[:, t * 64:(t + 1) * 64],
                                     start=(t == 0), stop=(t == 3))
                yq_sb = wpool.tile([128, 64], F32, name=f"yq_sb{tag}")
                nc.vector.tensor_scalar(yq_sb[:], yq_ps[:], rec[:], None,
                                        ALU.mult)
                return yq_sb

            yq1 = hopfield(yT[:], "h1")

            yqT_ps = psB.tile([64, 128], F32, tag="psB", name="yqT_ps")
            nc.tensor.transpose(yqT_ps[:], yq1[:], ident[:])
            r2T2 = wpool.tile([128, 128], F32)
            nc.vector.scalar_tensor_tensor(r2T2[0:64, :], yqT_ps[:], -1.0,
                                           yT[:], ALU.mult, ALU.add)
            nc.vector.tensor_mul(r2T2[0:64, :], r2T2[0:64, :], m2T[:])
            nc.vector.tensor_copy(r2T2[64:128, :], r2T2[0:64, :])

            # ---- Phase B: e_patch + argmin ----
            w1s16 = wpool.tile([32, 16], F32)
            nc.vector.tensor_reduce(
                w1s16[:],
                v(w1sb, 0, [[w1sb.ap[0][0], 32], [1, 16], [16, 3]]),
                AX.X, ALU.add)
            w1si = wpool.tile([32, 256], F32)
            nc.vector.memset(w1si[:], 0.0)
            nc.vector.tensor_copy(
                v(w1si, 102, [[w1si.ap[0][0], 32], [16, 4], [1, 4]]),
                w1s16[:])
            # w1stp4[g] [128=(l,m),100]: w1s-tap for k2=4g+l
            w1stp4 = []
            for g in range(4):
                wt = wpool.tile([128, 100], F32, name=f"w1stp4{g}")
                for l in range(4):
                    nc.vector.tensor_copy(
                        v(wt, 32 * l * wt.ap[0][0],
                          [[wt.ap[0][0], 32], [10, 10], [1, 10]]),
                        v(w1si, (6 - 2 * g) * 16 + 6 - 2 * l,
                          [[w1si.ap[0][0], 32], [16, 10], [1, 10]]))
                w1stp4.append(wt)

            # g1: 16 plain matmuls (K=64, alternating halves), masked into
            # the K-stacked g1m4[g] tiles consumed by ep/…
            g1m4 = [wpool.tile([128, 128], F32, name=f"g1m4{g}")
                    for g in range(4)]
            mtap = [[ypitch, 32], [324, 2], [36, 8], [2, 8]]
            for g in range(4):
                for l in range(4):
                    k2 = 4 * g + l
                    h = k2 % 2
                    g1ps = psB.tile([32, 128], F32, tag="psB",
                                    name=f"g1ps{g}_{l}")
                    nc.tensor.matmul(
                        g1ps[:],
                        v(w2c2s, 64 * h * w2c2s.ap[0][0] + k2,
                          [[w2c2s.ap[0][0], 64], [16, 32]]),
                        r2T2[64 * h:64 * (h + 1), :],
                        start=True, stop=True)
                    nc.vector.tensor_tensor(
                        g1m4[g][32 * l:32 * (l + 1), :],
                        g1ps[:],
                        v(m1p4, 32 * l * ypitch + 18 * g, mtap),
                        ALU.mult)

            ep_ps = psA.tile([100, 128], F32, tag="psA", name="ep_ps")
            for g in range(4):
                nc.tensor.matmul(ep_ps[:], w1stp4[g][:], g1m4[g][:],
                                 start=(g == 0), stop=(g == 3))
            ep_sb = wpool.tile([100, 128], F32)
            nc.scalar.copy(ep_sb[:], ep_ps[:])
            ep2_ps = psB.tile([128, 100], F32, tag="psB", name="ep2_ps")
            nc.tensor.transpose(ep2_ps[:], ep_sb[:], ident[0:100, 0:100])
            ep2 = wpool.tile([128, 100], F32)
            nc.scalar.copy(ep2[:], ep2_ps[:])

            # scatter patches to DRAM (padded per-o layout), gather E9 rows
            for b in range(2):
                for k in range(8):
                    r_, h_ = k // 2, k % 2
                    e_ = 1 if r_ >= 1 else 0
                    off = (b * 128000 + k * 16000
                           + (2 - e_) * 1600 + (2 - 4 * h_) * 100)
                    eng = dma if k % 2 == 0 else nc.scalar.dma_start
                    eng(v(edR, off, [[1600, 8], [100, 8], [1, 100]]),
                        ep2[b * 64:(b + 1) * 64, :])

            # data_e broadcast: 2 block loads + 4 doubling copies on two
            # DMA FIFOs (sync + scalar) instead of 8 serialized full loads
            data_e = wpool.tile([128, 4800], F32)
            dma(data_e[0:64, :],
                v(edR, 0, [[16000, 8], [1600, 8], [1, 4800]]))
            nc.scalar.dma_start(
                data_e[64:128, :],
                v(edR, 128000, [[16000, 8], [1600, 8], [1, 4800]]))

            e9 = wpool.tile([128, 144], F32)
            e9pitch = e9.ap[0][0]
            nc.gpsimd.indirect_copy(
                v(e9, 0, [[e9pitch, 128], [1, 144], [1, 1]]),
                data_e[:], ixE[:], True)

            # argmin with reference tie semantics
            mincand = wpool.tile([128, 16], F32)
            nc.vector.tensor_reduce(
                mincand[:], v(e9, 0, [[e9pitch, 128], [9, 16], [1, 9]]),
                AX.X, ALU.min)
            mstar = wpool.tile([128, 16], F32)
            nc.vector.tensor_scalar(mstar[:], mincand[:], 0.0, None, ALU.min)
            eq9 = wpool.tile([128, 144], F32)
            nc.vector.tensor_tensor(
                v(eq9, 0, [[eq9.ap[0][0], 128], [9, 16], [1, 9]]),
                v(e9, 0, [[e9pitch, 128], [9, 16], [1, 9]]),
                v(mstar, 0, [[mstar.ap[0][0], 128], [1, 16], [0, 9]]),
                ALU.is_equal)
            cs = wpool.tile([128, 144], F32)
            nc.vector.scalar_tensor_tensor(cs[:], eq9[:], -1000.0, oidx9[:],
                                           ALU.mult, ALU.add)
            minc2 = wpool.tile([128, 16], F32)
            nc.vector.tensor_reduce(
                minc2[:], v(cs, 0, [[cs.ap[0][0], 128], [9, 16], [1, 9]]),
                AX.X, ALU.min)
            zeq = wpool.tile([128, 16], F32)
            nc.vector.tensor_scalar(zeq[:], mstar[:], 0.0, None,
                                    ALU.is_equal)
            zsc = wpool.tile([128, 16], F32)
            nc.vector.scalar_tensor_tensor(zsc[:], zeq[:], -1000.0, zct[:],
                                           ALU.mult, ALU.add)
            sel16 = wpool.tile([128, 16], F32)
            nc.vector.tensor_tensor(sel16[:], minc2[:], zsc[:], ALU.min)
            nc.vector.tensor_scalar(sel16[:], sel16[:], 1000.0, None,
                                    ALU.add)

            # sel -> padded DRAM image (pad = -1, pre-filled)
            for b in range(2):
                for r in range(4):
                    eng = dma if r % 2 == 0 else nc.scalar.dma_start
                    eng(v(sel_pad, 1444 * b + 117 + 38 * r,
                          [[16, 2], [152, 8], [1, 16]]),
                        sel16[b * 64 + r * 16:b * 64 + r * 16 + 16, :])

            # ---- Phase C: sel gather + mask expansion ----
            # data_s broadcast: 1 16-row load + 3 doubling copies
            data_s = wpool.tile([128, 2888], F32)
            for i in range(4):
                eng = dma if i % 2 == 0 else nc.scalar.dma_start
                eng(data_s[16 * i:16 * (i + 1), :],
                    v(sel_pad, 0, [[38, 4], [1, 4], [1, 2888]]))
            dma(data_s[64:128, :], data_s[0:64, :])
            selm2 = []
            for s in range(2):
                sg = wpool.tile([128, 128], F32, name=f"sg{s}")
                nc.gpsimd.indirect_copy(
                    v(sg, 0, [[sg.ap[0][0], 128], [1, 128], [1, 1]]),
                    data_s[:], ixS[:, s * 8:(s + 1) * 8], True)
                nc.vector.tensor_tensor(sg[:], sg[:], oidx[:], ALU.is_equal)
                selm2.append(sg)

            xsel = []
            for t in range(8):
                mx_ps = psB.tile([128, 128], F32, tag="psB", name=f"mx{t}")
                nc.tensor.matmul(mx_ps[:], emt[:, t * 128:(t + 1) * 128],
                                 selm2[(t // 2) // 2][:],
                                 start=True, stop=True)
                xs = wpool.tile([128, 128], F32, name=f"xs{t}")
                nc.vector.tensor_tensor(xs[:], xg3[t][:], mx_ps[:],
                                        ALU.mult)
                xsel.append(xs)

            # z: 16 plain matmuls, masked into K-stacked zm4[g] tiles
            zm4 = [wpool.tile([128, 128], F32, name=f"zm4{g}")
                   for g in range(4)]
            for g in range(4):
                for l in range(4):
                    k2 = 4 * g + l
                    k2y, k2x = k2 // 4, k2 % 4
                    t = k2y * 2 + k2x // 2
                    half = (k2x % 2) * 64
                    zps = psB.tile([32, 128], F32, tag="psB",
                                   name=f"zps{g}_{l}")
                    nc.tensor.matmul(
                        zps[:],
                        w1fp[half:half + 64, :],
                        xsel[t][half:half + 64, :],
                        start=True, stop=True)
                    nc.vector.tensor_tensor(
                        zm4[g][32 * l:32 * (l + 1), :],
                        zps[:],
                        v(m1p4, 32 * l * ypitch + 18 * g, mtap),
                        ALU.mult)

            # ym: 4 K=128 accumulating matmuls
            ym_ps = psA.tile([128, 64], F32, tag="psA", name="ym_ps")
            for g in range(4):
                nc.tensor.matmul(
                    ym_ps[:],
                    zm4[g][:],
                    v(w2s4, 64 * g, [[w2s4.ap[0][0], 128], [1, 64]]),
                    start=(g == 0), stop=(g == 3))

            yTT_ps = psB.tile([128, 64], F32, tag="psB", name="yTT_ps")
            nc.tensor.transpose(yTT_ps[:], yT[:], ident[0:64, 0:64])
            m2g = wpool.tile([128, 64], F32)
            nc.vector.tensor_scalar(m2g[:], yTT_ps[:], 0.0, None, ALU.is_gt)
            ymm = wpool.tile([128, 64], F32)
            nc.vector.tensor_tensor(ymm[:], ym_ps[:], m2g[:], ALU.mult)

            t2_ps = psB.tile([64, 128], F32, tag="psB", name="t2_ps")
            nc.tensor.transpose(t2_ps[:], ymm[:], ident[:])
            ymmT = wpool.tile([64, 128], F32)
            nc.scalar.copy(ymmT[:], t2_ps[:])

            yq2 = hopfield(ymmT[:], "h2")

            tr_ps = psB.tile([64, 128], F32, tag="psB", name="tr_ps")
            nc.tensor.transpose(tr_ps[:], yq2[:], ident[:])
            outT = wpool.tile([64, 128], F32)
            nc.scalar.copy(outT[:], tr_ps[:])
            for b in range(2):
                dma(AP(out_d, b * 4096, [[64, 64], [8, 8], [1, 8]]),
                    outT[:, b * 64:(b + 1) * 64])

    return nc


_CACHE = {}


def kernel(**inputs) -> np.ndarray:
    from concourse.bass_utils import run_bass_kernel_spmd
    if "nc" not in _CACHE:
        from concourse import bacc
        nc = bacc.Bacc("TRN2", target_bir_lowering=False, debug=False,
                       num_devices=N_CORES)
        build_program(nc)
        nc.compile()
        _CACHE["nc"] = nc
        _CACHE["consts"] = _consts()
    nc = _CACHE["nc"]
    feed = {k: np.ascontiguousarray(np.asarray(val, np.float32))
            for k, val in inputs.items()}
    for k, val in _CACHE["consts"].items():
        feed[k] = val
    in_maps = [dict(feed) for _ in range(N_CORES)]
    res = run_bass_kernel_spmd(nc, in_maps, list(range(N_CORES)))
    return np.asarray(res.results[0]["out"], np.float32)



# revision 9
# speedup vs baseline: 1.3937x; 1.3937x over previous
"""Trainium2 Bass kernel for nn_Block1_87144886436577 (vq_codebook).

v3: eliminates both DRAM broadcast round trips of v2.

- e9 candidate table: ep patches are column-shuffled on-chip into
  dy-triple rows (ep3r), scattered to a small DRAM buffer edR2 (276KB
  vs 1MB+2.4MB), loaded back as per-partition 3-row windows (1296 f32)
  and gathered with a group-uniform index table.
- sel masks: sel image lives in a tiny DRAM buffer; a per-partition
  34x10 window broadcast (348KB vs 1.5MB) + one gather produces
  sel_patch[(b,o), 10x10]; one is_equal + one PE transpose gives
  S_cmp[(cy,cx), (b,o)]; the per-tap masks are then 8 constant
  permutation matmuls (emtX) in bf16 -- no data_s, no second gather
  pass, no 7.5us gpsimd drain.
- hopfield: no max-subtraction (logits are +-1), attention computed
  directly in [k, pix] orientation (no transposes), V|ones matmul
  yields sums for free, normalization broadcast via a K=1 matmul.
  Output hopfield + the whole phase-C (masks, z, ym) run in bf16.
- g1 collapsed 16->4 matmuls (lhsT = PE-transposed w2s4 blocks),
  z collapsed 16->8 (block-diagonal w1 lhsT).

Single-core program; all 8 cores run identical replicas. Output read
from core 0.
"""
import sys

import numpy as np

for _p in ("/opt/trn_rl_repo",):
    if _p not in sys.path:
        sys.path.insert(0, _p)

import concourse.bass as bass
import concourse.mybir as mybir
import concourse.tile as tile

F32 = mybir.dt.float32
BF16 = mybir.dt.bfloat16
U16 = mybir.dt.uint16
AF = mybir.ActivationFunctionType
ALU = mybir.AluOpType
AX = mybir.AxisListType
AP = bass.AP

N_CORES = 8


def v(t, off, pat):
    return AP(t.tensor, t.offset + off, pat)


def _e(r):
    return 1 if r >= 1 else 0


def _consts():
    """Host-precomputed constant tensors (input-independent)."""
    import ml_dtypes

    ident128 = np.eye(128, dtype=np.float32)

    # x gather idx (same as v2): value = element index into the
    # per-partition padded image row of data_x.
    idxX = np.zeros((8, 128, 8), np.uint16)
    for t in range(8):
        k2y, k2xh = t // 2, t % 2
        for g in range(8):
            k2xp = g // 4
            k1y = g % 4
            k2x = 2 * k2xh + k2xp
            for j in range(128):
                b, oy, ox = j // 64, (j % 64) // 8, j % 8
                idxX[t, 16 * g + j % 16, j // 16] = (
                    b * 1444 + (4 * oy + 2 * k2y + k1y) * 38
                    + 4 * ox + 2 * k2x)
    idxXs = np.ascontiguousarray(
        idxX.transpose(1, 0, 2).reshape(128, 64))

    # e9 gather idx into the 1296-wide data_e2 row: group-uniform.
    idxE3 = np.zeros((128, 9), np.uint16)
    for j in range(144):
        ixl, jj = j // 9, j % 9
        jy, jx = jj // 3, jj % 3
        sp, t = ixl // 4, ixl % 4
        et = 1 if t >= 1 else 0
        oxrel = sp + et - jx + 2
        dx = t - 4 * et + 4 * jx + 3
        val = (2 - jy) * 432 + oxrel * 36 + jy * 12 + dx
        idxE3[j % 16:128:16, j // 16] = val

    # sel_patch gather idx into the 680-wide data_sp row: per-group
    # (b, oyh) base + (8*oyh + dy)*10 + dx.
    idxSP = np.zeros((128, 7), np.uint16)
    for g in range(8):
        b, oyh = g // 4, g % 4
        for j in range(100):
            dy, dx = j // 10, j % 10
            val = b * 340 + (8 * oyh + dy) * 10 + dx
            idxSP[16 * g + j % 16, j // 16] = val

    # argmin helper tables (identical to v2)
    oidx9 = np.full((128, 144), 3000.0, np.float32)
    zc = np.zeros((128, 16), np.float32)
    for r in range(4):
        for h in range(2):
            for b in range(2):
                for q in range(8):
                    p = b * 64 + r * 16 + h * 8 + q
                    iy = 4 * q + r
                    for ixl in range(16):
                        ix = 16 * h + ixl
                        t_ = ix % 4
                        s = ix // 4
                        for jj in range(9):
                            jy, jx = jj // 3, jj % 3
                            oy = q + _e(r) - jy
                            ox = s + _e(t_) - jx
                            dy = iy - 4 * oy + 3
                            dx = ix - 4 * ox + 3
                            if (0 <= oy < 8 and 0 <= ox < 8
                                    and 0 <= dy < 10 and 0 <= dx < 10):
                                oidx9[p, ixl * 9 + jj] = oy * 8 + ox
                        for o in range(64):
                            oy, ox = o // 8, o % 8
                            if not (0 <= iy - 4 * oy + 3 < 10
                                    and 0 <= ix - 4 * ox + 3 < 10):
                                zc[p, ixl] = float(o)
                                break

    # emtX[t]: constant permutation: S_cmp row c=(10*cy+cx) -> mask
    # row p=(k2xp, k1y, ci, k1x) for tap tile t=(k2y, k2xh).
    emX = np.zeros((8, 128, 128), np.float32)
    for t in range(8):
        k2y, k2xh = t // 2, t % 2
        for p in range(128):
            k2xp = p // 64
            k1y = (p // 16) % 4
            k1x = p % 4
            k2x = 2 * k2xh + k2xp
            cy = 2 * k2y + k1y
            cx = 2 * k2x + k1x
            emX[t, 10 * cy + cx, p] = 1.0
    emtX = np.ascontiguousarray(
        emX.transpose(1, 0, 2).reshape(128, 1024)).astype(ml_dtypes.bfloat16)

    # per-partition output index o = p % 64 (for sel_patch compare)
    oidxP = (np.arange(128) % 64).astype(np.float32).reshape(128, 1)

    return {"ident128": ident128, "idxXs": idxXs, "idxE3": idxE3,
            "idxSP": idxSP, "oidx9": oidx9, "zc": zc, "emtX": emtX,
            "oidxP": oidxP}


def build_program(nc, debug=False):
    x_d = nc.declare_dram_parameter("x", [2, 3, 32, 32], F32, isOutput=False)
    w1_d = nc.declare_dram_parameter("w1", [32, 3, 4, 4], F32, isOutput=False)
    b1_d = nc.declare_dram_parameter("b1", [32], F32, isOutput=False)
    w2_d = nc.declare_dram_parameter("w2", [64, 32, 4, 4], F32, isOutput=False)
    b2_d = nc.declare_dram_parameter("b2", [64], F32, isOutput=False)
    k_d = nc.declare_dram_parameter("K", [512, 64], F32, isOutput=False)
    v_d = nc.declare_dram_parameter("V", [512, 64], F32, isOutput=False)
    id_d = nc.declare_dram_parameter("ident128", [128, 128], F32,
                                     isOutput=False)
    ixx_d = nc.declare_dram_parameter("idxXs", [128, 64], U16, isOutput=False)
    ixe_d = nc.declare_dram_parameter("idxE3", [128, 9], U16, isOutput=False)
    ixsp_d = nc.declare_dram_parameter("idxSP", [128, 7], U16, isOutput=False)
    oi9_d = nc.declare_dram_parameter("oidx9", [128, 144], F32,
                                      isOutput=False)
    zc_d = nc.declare_dram_parameter("zc", [128, 16], F32, isOutput=False)
    emx_d = nc.declare_dram_parameter("emtX", [128, 1024], BF16,
                                      isOutput=False)
    oip_d = nc.declare_dram_parameter("oidxP", [128, 1], F32, isOutput=False)
    out_d = nc.declare_dram_parameter("out", [2, 64, 8, 8], F32,
                                      isOutput=True)
    dbg = {}
    if debug:
        for nm, sh in [("d_yT", [64, 128]), ("d_r2T", [64, 128]),
                       ("d_ep2pad", [128, 144]), ("d_e9", [128, 144]),
                       ("d_sel16", [128, 16]), ("d_selpatch", [128, 100]),
                       ("d_Scmp", [128, 128]), ("d_ymmT", [64, 128]),
                       ("d_m1p4", [128, 648]), ("d_dataE", [128, 1296])]:
            dbg[nm] = nc.declare_dram_parameter(nm, sh, F32, isOutput=True)

    with tile.TileContext(nc) as tc:
        with (
            tc.tile_pool(name="const", bufs=1) as cpool,
            tc.tile_pool(name="work", bufs=1) as wpool,
            tc.tile_pool(name="psA", bufs=2, space="PSUM") as psA,
            tc.tile_pool(name="psB", bufs=4, space="PSUM") as psB,
            tc.tile_pool(name="psC", bufs=2, space="PSUM") as psC,
            tc.tile_pool(name="dram", bufs=1, space="DRAM") as dpool,
        ):
            dma = nc.sync.dma_start
            sdma = nc.scalar.dma_start
            gdma = nc.gpsimd.dma_start

            # ---- DRAM scratch ----
            x_pad2 = dpool.tile([11680], F32)      # [ci][b][38x38] + slop
            selD = dpool.tile([3040], F32)         # [b][38x40] sel image
            edR2 = dpool.tile([69120], F32)        # [b][k=(r,h)][10][12][36]

            # ---- critical-path staging on sync queue ----
            ident = cpool.tile([128, 128], F32)
            dma(ident[:], id_d[:])
            w1sb = wpool.tile([32, 48], F32)          # [m, (ci,k1)]
            dma(w1sb[:], AP(w1_d, 0, [[48, 32], [1, 48]]))
            xp4 = wpool.tile([128, 2888], F32)
            xpp = xp4.ap[0][0]
            nc.vector.memset(xp4[:], 0.0)
            for l in range(4):
                for b in range(2):
                    eng = dma if (2 * l + b) % 2 == 0 else sdma
                    eng(v(xp4, 32 * l * xpp + b * 1444 + 117 - l,
                          [[xpp, 3], [38, 32], [1, 32]]),
                        AP(x_d, b * 3072, [[1024, 3], [32, 32], [1, 32]]))

            # w1 tap-stacked lhsT for conv1: one transpose of IN where
            # IN[32*k1y + m, 32*k1x + ci] = w1[m, ci, k1y, k1x]
            w1IN = wpool.tile([128, 128], F32)
            nc.vector.memset(w1IN[:], 0.0)
            wip = w1IN.ap[0][0]
            for k1y in range(4):
                nc.vector.tensor_copy(
                    v(w1IN, 32 * k1y * wip,
                      [[wip, 32], [32, 4], [1, 3]]),
                    v(w1sb, 4 * k1y, [[w1sb.ap[0][0], 32], [1, 4], [16, 3]]))
            wg_ps = psB.tile([128, 128], F32, tag="psB", name="wg_ps")
            nc.tensor.transpose(wg_ps[:], w1IN[:], ident[:])
            w1gall = wpool.tile([128, 128], F32)
            nc.scalar.copy(w1gall[:], wg_ps[:])

            # ---- secondary staging (scalar/vector queues) ----
            ixX = cpool.tile([128, 64], U16)
            sdma(ixX[:], ixx_d[:])
            ixE = cpool.tile([128, 9], U16)
            sdma(ixE[:], ixe_d[:])
            ixSP = cpool.tile([128, 7], U16)
            sdma(ixSP[:], ixsp_d[:])
            oidx9 = cpool.tile([128, 144], F32)
            sdma(oidx9[:], oi9_d[:])
            zct = cpool.tile([128, 16], F32)
            sdma(zct[:], zc_d[:])
            emtX = cpool.tile([128, 1024], BF16)
            sdma(emtX[:], emx_d[:])
            oidxP = cpool.tile([128, 1], F32)
            sdma(oidxP[:], oip_d[:])

            # zero fills
            zx = cpool.tile([16, 2336], F32)
            nc.vector.memset(zx[:], 0.0)
            gdma(v(x_pad2, 0, [[2336, 5], [1, 2336]]), zx[0:5, :])
            zf = cpool.tile([128, 540], F32)
            nc.vector.memset(zf[:], 0.0)
            gdma(v(edR2, 0, [[540, 128], [1, 540]]), zf[:])
            selDf = cpool.tile([2, 1520], F32)
            nc.vector.memset(selDf[:], -1.0)
            gdma(v(selD, 0, [[1520, 2], [1, 1520]]), selDf[:])

            # padded x image in DRAM (feeds data_x broadcast)
            for b in range(2):
                gdma(v(x_pad2, b * 1444 + 117,
                       [[2888, 3], [38, 32], [1, 32]]),
                     AP(x_d, b * 3072, [[1024, 3], [32, 32], [1, 32]]))

            # conv2 weights: w2sb [m, (c,k2)] then w2s4 [(l,m), (g,c)]
            w2sb = wpool.tile([32, 1024], F32)
            sdma(w2sb[:], AP(w2_d, 0, [[16, 32], [512, 64], [1, 16]]))
            w2s4 = wpool.tile([128, 256], F32)
            for l in range(4):
                nc.vector.tensor_copy(
                    v(w2s4, 32 * l * w2s4.ap[0][0],
                      [[w2s4.ap[0][0], 32], [64, 4], [1, 64]]),
                    v(w2sb, l, [[w2sb.ap[0][0], 32], [4, 4], [16, 64]]))
            # bf16 copy for the ym contraction
            w2s4b = wpool.tile([128, 256], BF16)
            nc.vector.tensor_copy(w2s4b[:], w2s4[:])
            # g1 lhsT: transpose each w2s4 g-block -> [64 c, 128 (l,m)]
            g1L = []
            for g in range(4):
                gt_ps = psB.tile([64, 128], F32, tag="psB", name=f"g1Lp{g}")
                nc.tensor.transpose(gt_ps[:], w2s4[:, 64 * g:64 * (g + 1)],
                                    ident[:])
                gl = wpool.tile([64, 128], F32, name=f"g1L{g}")
                nc.scalar.copy(gl[:], gt_ps[:])
                g1L.append(gl)

            b1t4 = wpool.tile([128, 1], F32)
            sdma(b1t4[0:32, :], AP(b1_d, 0, [[1, 32], [1, 1]]))
            sdma(b1t4[32:64, :], b1t4[0:32, :])
            sdma(b1t4[64:128, :], b1t4[0:64, :])
            b2t = wpool.tile([64, 1], F32)
            sdma(b2t[:], AP(b2_d, 0, [[1, 64], [1, 1]]))

            # K^T (scaled by beta=0.125) and V|ones in SBUF
            k4 = wpool.tile([128, 256], F32)
            sdma(k4[:], AP(k_d, 0, [[64, 128], [8192, 4], [1, 64]]))
            kt_sb = wpool.tile([64, 512], F32)
            for t in range(4):
                kt_ps = psB.tile([64, 128], F32, tag="psB", name=f"ktp{t}")
                nc.tensor.transpose(kt_ps[:], k4[:, 64 * t:64 * (t + 1)],
                                    ident[:])
                nc.scalar.activation(kt_sb[:, 128 * t:128 * (t + 1)],
                                     kt_ps[:], AF.Copy, scale=0.125)
            ktb = wpool.tile([64, 512], BF16)
            nc.vector.tensor_copy(ktb[:], kt_sb[:])
            v1_sb = wpool.tile([128, 260], F32)
            sdma(v(v1_sb, 0, [[v1_sb.ap[0][0], 128], [65, 4], [1, 64]]),
                 AP(v_d, 0, [[64, 128], [8192, 4], [1, 64]]))
            nc.vector.memset(
                v(v1_sb, 64, [[v1_sb.ap[0][0], 128], [65, 4], [1, 1]]), 1.0)
            v1b = wpool.tile([128, 260], BF16)
            nc.vector.tensor_copy(v1b[:], v1_sb[:])
            ones64 = cpool.tile([1, 64], F32)
            nc.vector.memset(ones64[:], 1.0)

            # misc pre-zeroed tiles
            ep2pad = wpool.tile([128, 144], F32)
            nc.vector.memset(ep2pad[:], 0.0)
            S_cmp = wpool.tile([128, 128], BF16)
            nc.vector.memset(S_cmp[:], 0.0)
            w1diag = wpool.tile([128, 64], BF16)
            nc.vector.memset(w1diag[:], 0.0)

            # w1fp: [64=(k1y,ci,k1x), 32 m] for the z contraction
            w1sb2 = wpool.tile([32, 48], F32)
            sdma(w1sb2[:], AP(w1_d, 0, [[48, 32], [16, 3], [1, 16]]))
            w1sb2p = wpool.tile([32, 64], F32)
            nc.vector.memset(w1sb2p[:], 0.0)
            nc.vector.tensor_copy(
                v(w1sb2p, 0, [[w1sb2p.ap[0][0], 32], [16, 4], [4, 3], [1, 4]]),
                v(w1sb2, 0, [[w1sb2.ap[0][0], 32], [4, 4], [16, 3], [1, 4]]))
            w1fp_ps = psB.tile([64, 32], F32, tag="psB", name="w1fp_ps")
            nc.tensor.transpose(w1fp_ps[:], w1sb2p[:], ident[0:32, 0:32])
            nc.scalar.copy(w1diag[0:64, 0:32], w1fp_ps[:])
            nc.scalar.copy(w1diag[64:128, 32:64], w1fp_ps[:])

            # x broadcast + gathers, self-contained on gpsimd (SWDGE)
            data_x = wpool.tile([128, 2888], F32)
            for ci in range(4):
                gdma(data_x[ci * 4:ci * 4 + 4, :],
                     v(x_pad2, ci * 2888, [[1, 4], [1, 2888]]))
            for d in (16, 32, 64):
                gdma(data_x[d:2 * d, :], data_x[0:d, :])
            xg3 = []
            for t in range(8):
                xg = wpool.tile([128, 128], F32, name=f"xg{t}")
                nc.gpsimd.indirect_copy(
                    v(xg, 0, [[xg.ap[0][0], 128], [1, 128], [1, 1]]),
                    data_x[:], ixX[:, t * 8:(t + 1) * 8], True)
                xg3.append(xg)

            # w1s taps for the e-patch matmuls
            w1s16 = wpool.tile([32, 16], F32)
            nc.vector.tensor_reduce(
                w1s16[:],
                v(w1sb, 0, [[w1sb.ap[0][0], 32], [1, 16], [16, 3]]),
                AX.X, ALU.add)
            w1si = wpool.tile([32, 256], F32)
            nc.vector.memset(w1si[:], 0.0)
            nc.vector.tensor_copy(
                v(w1si, 102, [[w1si.ap[0][0], 32], [16, 4], [1, 4]]),
                w1s16[:])
            w1stp4 = []
            for g in range(4):
                wt = wpool.tile([128, 100], F32, name=f"w1stp4{g}")
                for l in range(4):
                    nc.vector.tensor_copy(
                        v(wt, 32 * l * wt.ap[0][0],
                          [[wt.ap[0][0], 32], [10, 10], [1, 10]]),
                        v(w1si, (6 - 2 * g) * 16 + 6 - 2 * l,
                          [[w1si.ap[0][0], 32], [16, 10], [1, 10]]))
                w1stp4.append(wt)

            # ---- Phase A: forward ----
            y1ps = psA.tile([32, 512], F32, tag="psA", name="y1ps")
            for g in range(4):
                nc.tensor.matmul(
                    y1ps[:], w1gall[:, 32 * g:32 * (g + 1)],
                    v(xp4, 78 + 38 * g, [[xpp, 128], [1444, 2],
                                         [76, 16], [2, 16]]),
                    start=(g == 0), stop=(g == 3))
            y1p4 = wpool.tile([128, 648], F32)
            nc.vector.memset(y1p4[:], 0.0)
            ypitch = y1p4.ap[0][0]
            nc.scalar.activation(
                v(y1p4, 19, [[ypitch, 32], [324, 2], [18, 16], [1, 16]]),
                y1ps[:], AF.Relu, bias=b1t4[0:32, :])
            iview = [[ypitch, 32], [324, 2], [18, 16], [1, 16]]
            for l in range(1, 4):
                nc.vector.tensor_copy(
                    v(y1p4, 32 * l * ypitch + 19 - l, iview),
                    v(y1p4, 19, iview))
            m1p4 = wpool.tile([128, 648], F32)
            nc.vector.tensor_scalar(m1p4[:], y1p4[:], 0.0, None, ALU.is_gt)

            def tapg(tl, g, pitch, np_=128, base=0):
                return v(tl, base * pitch + 18 * g,
                         [[pitch, np_], [324, 2], [36, 8], [2, 8]])

            ypre = psA.tile([64, 128], F32, tag="psA", name="ypre")
            for g in range(4):
                nc.tensor.matmul(
                    ypre[:],
                    v(w2s4, 64 * g, [[w2s4.ap[0][0], 128], [1, 64]]),
                    tapg(y1p4, g, ypitch),
                    start=(g == 0), stop=(g == 3))
            yT = wpool.tile([64, 128], F32)    # [c, (b,o)]
            nc.scalar.activation(yT[:], ypre[:], AF.Relu, bias=b2t[:])
            m2T = wpool.tile([64, 128], F32)
            nc.vector.tensor_scalar(m2T[:], yT[:], 0.0, None, ALU.is_gt)

            # ---- hopfield 1 (fp32, argmin-critical) ----
            pt1 = []
            for t in range(4):
                pT_ps = psB.tile([128, 128], F32, tag="psB", name=f"pT1{t}")
                nc.tensor.matmul(pT_ps[:], kt_sb[:, 128 * t:128 * (t + 1)],
                                 yT[:], start=True, stop=True)
                pt = wpool.tile([128, 128], F32, name=f"pt1{t}")
                nc.scalar.activation(pt[:], pT_ps[:], AF.Exp)
                pt1.append(pt)
            yq_ps = psC.tile([65, 128], F32, tag="psC", name="yq1")
            for t in range(4):
                nc.tensor.matmul(yq_ps[:],
                                 v(v1_sb, 65 * t, [[v1_sb.ap[0][0], 128],
                                                   [1, 65]]),
                                 pt1[t][:], start=(t == 0), stop=(t == 3))
            rec = wpool.tile([1, 128], F32)
            nc.vector.reciprocal(rec[:], yq_ps[64:65, :])
            rb_ps = psB.tile([64, 128], F32, tag="psB", name="rb1")
            nc.tensor.matmul(rb_ps[:], ones64[:], rec[:], start=True,
                             stop=True)
            rb_sb = wpool.tile([64, 128], F32)
            nc.scalar.copy(rb_sb[:], rb_ps[:])
            r2T = wpool.tile([64, 128], F32)
            nc.vector.tensor_tensor(r2T[:], yq_ps[0:64, :], rb_sb[:],
                                    ALU.mult)
            nc.vector.scalar_tensor_tensor(r2T[:], r2T[:], -1.0, yT[:],
                                           ALU.mult, ALU.add)
            nc.vector.tensor_tensor(r2T[:], r2T[:], m2T[:], ALU.mult)

            # ---- Phase B: e-patch + argmin ----
            g1m4 = []
            for g in range(4):
                g1ps = psB.tile([128, 128], F32, tag="psB", name=f"g1ps{g}")
                nc.tensor.matmul(g1ps[:], g1L[g][:], r2T[:], start=True,
                                 stop=True)
                gm = wpool.tile([128, 128], F32, name=f"g1m4{g}")
                nc.vector.tensor_tensor(gm[:], g1ps[:],
                                        tapg(m1p4, g, ypitch), ALU.mult)
                g1m4.append(gm)

            ep_ps = psA.tile([100, 128], F32, tag="psA", name="ep_ps")
            for g in range(4):
                nc.tensor.matmul(ep_ps[:], w1stp4[g][:], g1m4[g][:],
                                 start=(g == 0), stop=(g == 3))
            ep_sb = wpool.tile([100, 128], F32)
            nc.scalar.copy(ep_sb[:], ep_ps[:])
            ep2_ps = psB.tile([128, 100], F32, tag="psB", name="ep2_ps")
            nc.tensor.transpose(ep2_ps[:], ep_sb[:], ident[0:100, 0:100])
            # padded copy: cols (dy, dx) on a 12-wide dx pitch, zeros at
            # dx in {10,11} and dy in {10,11}
            nc.scalar.copy(
                v(ep2pad, 0, [[ep2pad.ap[0][0], 128], [12, 10], [1, 10]]),
                ep2_ps[:])

            # dy-triple shuffles: ep3r[r][p, sub*12+dx] = ep2pad[p,
            # dy(r,sub)*12+dx], dy(r,sub) = r - 4*e(r) + 3 + 4*sub
            ep3r = []
            for r in range(4):
                dy0 = r - 4 * _e(r) + 3
                e3 = wpool.tile([128, 36], F32, name=f"ep3r{r}")
                nc.vector.tensor_copy(
                    e3[:],
                    v(ep2pad, dy0 * 12,
                      [[ep2pad.ap[0][0], 128], [48, 3], [1, 12]]))
                ep3r.append(e3)

            # scatter to edR2: per (b, r, h): rows oy+2-e, slots ox+2-4h
            nscat = 0
            for b in range(2):
                for r in range(4):
                    for h in range(2):
                        e_ = _e(r)
                        k = r * 2 + h
                        base = (b * 34560 + k * 4320 + (2 - e_) * 432
                                + (2 - 4 * h) * 36)
                        eng = (dma, sdma)[nscat % 2]
                        nscat += 1
                        eng(v(edR2, base, [[432, 8], [36, 8], [1, 36]]),
                            ep3r[r][b * 64:(b + 1) * 64, :])

            # gather windows: data_e2[p=(b,k,q)] = edR2[b][k] rows q..q+2
            data_e2 = wpool.tile([128, 1296], F32)
            dma(data_e2[0:64, :],
                v(edR2, 0, [[4320, 8], [432, 8], [1, 1296]]))
            sdma(data_e2[64:128, :],
                 v(edR2, 34560, [[4320, 8], [432, 8], [1, 1296]]))

            e9 = wpool.tile([128, 144], F32)
            e9pitch = e9.ap[0][0]
            nc.gpsimd.indirect_copy(
                v(e9, 0, [[e9pitch, 128], [1, 144], [1, 1]]),
                data_e2[:], ixE[:], True)

            # argmin with reference tie semantics (identical to v2)
            mincand = wpool.tile([128, 16], F32)
            nc.vector.tensor_reduce(
                mincand[:], v(e9, 0, [[e9pitch, 128], [9, 16], [1, 9]]),
                AX.X, ALU.min)
            mstar = wpool.tile([128, 16], F32)
            nc.vector.tensor_scalar(mstar[:], mincand[:], 0.0, None, ALU.min)
            eq9 = wpool.tile([128, 144], F32)
            nc.vector.tensor_tensor(
                v(eq9, 0, [[eq9.ap[0][0], 128], [9, 16], [1, 9]]),
                v(e9, 0, [[e9pitch, 128], [9, 16], [1, 9]]),
                v(mstar, 0, [[mstar.ap[0][0], 128], [1, 16], [0, 9]]),
                ALU.is_equal)
            cs = wpool.tile([128, 144], F32)
            nc.vector.scalar_tensor_tensor(cs[:], eq9[:], -1000.0, oidx9[:],
                                           ALU.mult, ALU.add)
            minc2 = wpool.tile([128, 16], F32)
            nc.vector.tensor_reduce(
                minc2[:], v(cs, 0, [[cs.ap[0][0], 128], [9, 16], [1, 9]]),
                AX.X, ALU.min)
            zeq = wpool.tile([128, 16], F32)
            nc.vector.tensor_scalar(zeq[:], mstar[:], 0.0, None,
                                    ALU.is_equal)
            zsc = wpool.tile([128, 16], F32)
            nc.vector.scalar_tensor_tensor(zsc[:], zeq[:], -1000.0, zct[:],
                                           ALU.mult, ALU.add)
            sel16 = wpool.tile([128, 16], F32)
            nc.vector.tensor_tensor(sel16[:], minc2[:], zsc[:], ALU.min)
            nc.vector.tensor_scalar(sel16[:], sel16[:], 1000.0, None,
                                    ALU.add)

            # ---- sel -> masks via patch trick ----
            # sel image scatter: selD[b][(4q+r+3)*40 + 16h+3 + ixl]
            nsc = 0
            for b in range(2):
                for r in range(4):
                    eng = (dma, sdma)[nsc % 2]
                    nsc += 1
                    eng(v(selD, b * 1520 + (r + 3) * 40 + 3,
                          [[16, 2], [160, 8], [1, 16]]),
                        sel16[b * 64 + 16 * r:b * 64 + 16 * (r + 1), :])
            # window broadcast: data_sp[p=(ys,ox mod 16)] =
            #   [b-slab 340 | b-slab 340], slab = selD[b][4ys:4ys+34, 4ox:+10]
            data_sp = wpool.tile([128, 680], F32)
            dsp = data_sp.ap[0][0]
            nwin = 0
            for b in range(2):
                for ys in range(2):
                    eng = (dma, sdma)[nwin % 2]
                    nwin += 1
                    eng(v(data_sp, ys * 8 * dsp + b * 340,
                          [[dsp, 8], [10, 34], [1, 10]]),
                        v(selD, b * 1520 + ys * 160,
                          [[4, 8], [40, 34], [1, 10]]))
            dma(data_sp[16:32, :], data_sp[0:16, :])
            sdma(data_sp[32:64, :], data_sp[0:32, :])
            dma(data_sp[64:128, :], data_sp[0:64, :])

            sel_patch = wpool.tile([128, 100], F32)
            nc.gpsimd.indirect_copy(
                v(sel_patch, 0, [[sel_patch.ap[0][0], 128], [1, 100], [1, 1]]),
                data_sp[:], ixSP[:], True)
            Cf = wpool.tile([128, 100], F32)
            nc.vector.tensor_tensor(
                Cf[:], sel_patch[:],
                v(oidxP, 0, [[oidxP.ap[0][0], 128], [0, 100]]),
                ALU.is_equal)
            ct_ps = psB.tile([100, 128], F32, tag="psB", name="ct_ps")
            nc.tensor.transpose(ct_ps[:], Cf[:], ident[:])
            nc.scalar.copy(S_cmp[0:100, :], ct_ps[:])

            # ---- Phase C: masks, z, ym (bf16) ----
            zm4 = [wpool.tile([128, 128], BF16, name=f"zm4{g}")
                   for g in range(4)]
            for t in range(8):
                g, k2xh = t // 2, t % 2
                mx_ps = psB.tile([128, 128], F32, tag="psB", name=f"mx{t}")
                nc.tensor.matmul(mx_ps[:], emtX[:, 128 * t:128 * (t + 1)],
                                 S_cmp[:], start=True, stop=True)
                xs = wpool.tile([128, 128], BF16, name=f"xs{t}")
                nc.vector.tensor_tensor(xs[:], xg3[t][:], mx_ps[:], ALU.mult)
                zps = psC.tile([64, 128], F32, tag="psC", name=f"zps{t}")
                nc.tensor.matmul(zps[:], w1diag[:], xs[:], start=True,
                                 stop=True)
                nc.vector.tensor_tensor(
                    zm4[g][64 * k2xh:64 * (k2xh + 1), :], zps[:],
                    tapg(m1p4, g, ypitch, np_=64, base=32 * (2 * k2xh)),
                    ALU.mult)

            ymT_ps = psA.tile([64, 128], F32, tag="psA", name="ymT_ps")
            for g in range(4):
                nc.tensor.matmul(
                    ymT_ps[:],
                    v(w2s4b, 64 * g, [[w2s4b.ap[0][0], 128], [1, 64]]),
                    zm4[g][:], start=(g == 0), stop=(g == 3))
            ymmT = wpool.tile([64, 128], BF16)
            nc.vector.tensor_tensor(ymmT[:], ymT_ps[:], m2T[:], ALU.mult)

            # ---- hopfield 2 (bf16) ----
            pt2 = []
            for t in range(4):
                pT_ps = psB.tile([128, 128], F32, tag="psB", name=f"pT2{t}")
                nc.tensor.matmul(pT_ps[:], ktb[:, 128 * t:128 * (t + 1)],
                                 ymmT[:], start=True, stop=True)
                pt = wpool.tile([128, 128], BF16, name=f"pt2{t}")
                nc.scalar.activation(pt[:], pT_ps[:], AF.Exp)
                pt2.append(pt)
            yq2_ps = psC.tile([65, 128], F32, tag="psC", name="yq2")
            for t in range(4):
                nc.tensor.matmul(yq2_ps[:],
                                 v(v1b, 65 * t, [[v1b.ap[0][0], 128],
                                                 [1, 65]]),
                                 pt2[t][:], start=(t == 0), stop=(t == 3))
            rec2 = wpool.tile([1, 128], F32)
            nc.vector.reciprocal(rec2[:], yq2_ps[64:65, :])
            rb2_ps = psB.tile([64, 128], F32, tag="psB", name="rb2")
            nc.tensor.matmul(rb2_ps[:], ones64[:], rec2[:], start=True,
                             stop=True)
            rb2_sb = wpool.tile([64, 128], F32)
            nc.scalar.copy(rb2_sb[:], rb2_ps[:])
            outT = wpool.tile([64, 128], F32)
            nc.vector.tensor_tensor(outT[:], yq2_ps[0:64, :], rb2_sb[:],
                                    ALU.mult)
            for b in range(2):
                eng = dma if b == 0 else sdma
                eng(AP(out_d, b * 4096, [[64, 64], [8, 8], [1, 8]]),
                    outT[:, b * 64:(b + 1) * 64])
            if debug:
                def dump(nm, t, rows, cols, cast=None):
                    if cast is not None:
                        tf = wpool.tile([rows, cols], F32, name=nm + "c")
                        nc.vector.tensor_copy(tf[:], t)
                        t = tf[:]
                    sdma(AP(dbg[nm], 0, [[cols, rows], [1, cols]]), t)
                dump("d_yT", yT[:], 64, 128)
                dump("d_r2T", r2T[:], 64, 128)
                dump("d_ep2pad", ep2pad[:], 128, 144)
                dump("d_e9", e9[:], 128, 144)
                dump("d_sel16", sel16[:], 128, 16)
                dump("d_selpatch", sel_patch[:], 128, 100)
                dump("d_Scmp", S_cmp[:], 128, 128, cast=1)
                dump("d_ymmT", ymmT[:], 64, 128, cast=1)
                dump("d_m1p4", m1p4[:], 128, 648)
                dump("d_dataE", data_e2[:], 128, 1296)

    return nc


_CACHE = {}


def kernel(**inputs) -> np.ndarray:
    from concourse.bass_utils import run_bass_kernel_spmd
    if "nc" not in _CACHE:
        from concourse import bacc
        nc = bacc.Bacc("TRN2", target_bir_lowering=False, debug=False,
                       num_devices=N_CORES)
        build_program(nc)
        nc.compile()
        _CACHE["nc"] = nc
        _CACHE["consts"] = _consts()
    nc = _CACHE["nc"]
    feed = {k: np.ascontiguousarray(np.asarray(val, np.float32))
            for k, val in inputs.items()}
    for k, val in _CACHE["consts"].items():
        feed[k] = val
    in_maps = [dict(feed) for _ in range(N_CORES)]
    res = run_bass_kernel_spmd(nc, in_maps, list(range(N_CORES)))
    return np.asarray(res.results[0]["out"], np.float32)
